# revision 13
# baseline (speedup 1.0000x reference)
"""
Trainium2 Bass kernel for nn_ALSTM_SIN (broken-recurrence LSTM + FC head).

Model (from the reference):
  - gate pre-activations depend ONLY on x (the hidden-state recurrence
    multiplies a zero vector, so w_h* / recurrent terms are exactly 0):
        g = tanh(w_ig[b] @ x_t + b_g),  i/f/o = sigmoid(w_i{i,f,o}[b] @ x_t + b_*)
  - cell scan: c_t = f_t * c_{t-1} + i_t * g_t  (elementwise over (B,H))
  - h_fin = o_{T-1} * tanh(c_{T-1});  out = log_softmax(tanh(h@fc1.T+b1)@fc2.T+b2)

Key numerical fact (verified against the reference, and guarded at run time
on the actual inputs): the suffix product of the forget gates over the last
KT=256 timesteps underflows fp32 for every (b,h) lane, so (a) the scan
truncated to the last KT steps is exact in fp32, and (b) chaining the 4
per-sample scans of a half-batch into one fused scan along the free dim is
also exact (the cross-sample carry decays by the same product).  Only o at
t=T-1 matters for the output.

log_softmax over 2 classes is computed as ln(sigmoid(+/-d)), d = z1 - z0,
which needs no exp / max-subtraction and keeps the ACT table-set switches
to a single Ln load at the end.

Sharding: data-parallel over the per-sample weight/batch dim: 8 samples per
core x 8 cores.  No collectives; host concatenates per-core outputs.

Gate matmuls run in float32r (TF32-like, ~13-bit mantissa, full PE rate at
moving dim >= 256); the tiny o-gate / FC-head matmuls stay fp32.

Per-core device layout (host pre-transposes so all DMAs are contiguous):
  xt [I=128, BL=8, KT]  x slice, partition = input feature i
  wg/wi/wf/wo [I=128, BL=8, H=128]   w[b].T blocks (lhsT for matmul)
  fc [128, 40]: cols 0:32 fc1_w.T | col 32 fc1_b | cols 33:35 +/-dw (lhsT
     for the +/-d matmul) | col 35 +/-db bias | col 36 scratch
  out [O=2, BL=8] = log_softmax.T (host transposes back)
"""

import os
import sys

import numpy as np

sys.path.insert(0, "/opt/trn_rl_repo")

import concourse.bacc as bacc
import concourse.bass as bass
import concourse.mybir as mybir
import concourse.tile as tile

T, B, I, H, FC, O = 2048, 64, 128, 128, 32, 2
NCORES = 8
BL = B // NCORES            # samples per core
KT = 256                    # truncated timesteps (see header note)
AF = mybir.ActivationFunctionType
ALU = mybir.AluOpType
F32 = mybir.dt.float32
F32R = mybir.dt.float32r    # fp32 bits; PE rounds to ~13-bit mantissa
BF16 = mybir.dt.bfloat16
N_WARM = 10                 # PE warm-up matmuls (HAM un-throttle)


def build_nc(has_bias: bool, kt: int = KT):
    nc = bacc.Bacc(None, target_bir_lowering=False)

    xt = nc.dram_tensor("xt", [I, BL, kt], F32R, kind="ExternalInput")
    wg = nc.dram_tensor("wg", [I, BL, H], F32R, kind="ExternalInput")
    wi = nc.dram_tensor("wi", [I, BL, H], F32R, kind="ExternalInput")
    wf = nc.dram_tensor("wf", [I, BL, H], F32R, kind="ExternalInput")
    wo = nc.dram_tensor("wo", [I, BL, H], F32R, kind="ExternalInput")
    fcp = nc.dram_tensor("fc", [H, 40], F32, kind="ExternalInput")
    bias = None
    if has_bias:
        bias = nc.dram_tensor("bias", [H, 4 * BL], F32, kind="ExternalInput")
    out = nc.dram_tensor("out", [O, BL], F32, kind="ExternalOutput")

    SPH = BL // 2          # samples per half
    HW = SPH * kt          # free width of one half

    with tile.TileContext(nc) as tc:
        with (
            tc.tile_pool(name="inp", bufs=1) as inp,
            tc.tile_pool(name="gates", bufs=1) as gates,
            tc.tile_pool(name="small", bufs=1) as small,
            tc.tile_pool(name="psum_big", bufs=3, space="PSUM") as psb,
            tc.tile_pool(name="psum_small", bufs=2, space="PSUM") as pss,
        ):
            # ---- PE warm-up: dummy bf16 matmuls on a zero tile so the HAM
            # clock gate opens while the input DMAs stream ----
            wz = small.tile([H, 512], BF16, tag="wz", name="wz")
            nc.gpsimd.memset(wz[:], 0.0)
            for wi_ in range(N_WARM):
                wps = pss.tile([H, 512], F32, tag="sps", name="warm_ps")
                nc.tensor.matmul(wps[:], wz[:, 0:H], wz[:], start=True,
                                 stop=True)

            x_sb = inp.tile([I, BL * kt], F32R, tag="x", name="x_sb")
            w_sb = {}
            for name, t in (("g", wg), ("i", wi), ("f", wf), ("o", wo)):
                w_sb[name] = inp.tile([I, BL * H], F32R, tag=f"w{name}",
                                      name=f"w{name}_sb")
            fc_sb = small.tile([H, 40], F32, tag="fc", name="fc_sb")
            bias_sb = None
            if has_bias:
                bias_sb = small.tile([H, 4 * BL], F32, tag="bias",
                                     name="bias_sb")
                nc.sync.dma_start(bias_sb[:], bias[:])

            # DMAs ordered so the first quarter's gate matmuls unblock ASAP.
            xflat = xt[:].rearrange("i b t -> i (b t)")
            wflat = {k: w.rearrange("i b h -> i (b h)")
                     for k, w in ((k, t[:]) for k, t in
                                  (("g", wg), ("i", wi), ("f", wf), ("o", wo)))}
            # DMA issues spread across three engines (issue itself costs
            # ~0.6us per DMA on the issuing sequencer): sync feeds half 0,
            # vector feeds half 1, gpsimd brings the stragglers.
            QW = HW // 2   # 2-sample x chunk
            for half, eng in ((0, nc.sync), (1, nc.gpsimd)):
                wlo, whi = half * SPH * H, (half + 1) * SPH * H
                xlo = half * HW
                eng.dma_start(w_sb["i"][:, wlo:whi], wflat["i"][:, wlo:whi])
                eng.dma_start(x_sb[:, xlo:xlo + QW], xflat[:, xlo:xlo + QW])
                eng.dma_start(x_sb[:, xlo + QW:xlo + HW],
                              xflat[:, xlo + QW:xlo + HW])
                eng.dma_start(w_sb["g"][:, wlo:whi], wflat["g"][:, wlo:whi])
                eng.dma_start(w_sb["f"][:, wlo:whi], wflat["f"][:, wlo:whi])
            nc.scalar.dma_start(w_sb["o"][:], wflat["o"])
            nc.scalar.dma_start(fc_sb[:], fcp[:])

            g_sb = gates.tile([H, BL * kt], F32, tag="g", name="g_sb")
            i_sb = gates.tile([H, BL * kt], F32, tag="i", name="i_sb")
            f_sb = gates.tile([H, BL * kt], F32, tag="f", name="f_sb")
            u_sb = gates.tile([H, BL * kt], F32, tag="u", name="u_sb")
            c_sb = gates.tile([H, BL * kt], F32, tag="c", name="c_sb")

            # Gate order (i, g, f): the first ACT op is a Sigmoid, so walrus
            # loads sigmoid_and_others (which also has Tanh) once; u = i*g
            # runs on DVE in parallel with f's activation on ACT.
            gate_cfg = [("i", i_sb, AF.Sigmoid, 1), ("g", g_sb, AF.Tanh, 0),
                        ("f", f_sb, AF.Sigmoid, 2)]
            for half in range(2):
                lo = half * HW
                for name, dst, func, gidx in gate_cfg:
                    ps = psb.tile([H, HW], F32, tag="gate_ps", name="gate_ps")
                    for s in range(SPH):
                        b = half * SPH + s
                        nc.tensor.matmul(
                            ps[:, s * kt:(s + 1) * kt],
                            w_sb[name][:, b * H:(b + 1) * H],
                            x_sb[:, b * kt:(b + 1) * kt],
                            start=True, stop=True,
                        )
                    if has_bias:
                        for s in range(SPH):
                            b = half * SPH + s
                            nc.scalar.activation(
                                dst[:, lo + s * kt: lo + (s + 1) * kt],
                                ps[:, s * kt:(s + 1) * kt],
                                func,
                                bias=bias_sb[:, gidx * BL + b: gidx * BL + b + 1],
                            )
                    else:
                        nc.scalar.activation(dst[:, lo: lo + HW], ps[:], func)
                    if name == "g":
                        nc.vector.tensor_mul(
                            u_sb[:, lo: lo + HW],
                            i_sb[:, lo: lo + HW],
                            g_sb[:, lo: lo + HW],
                        )
                # One fused scan per half: the carry leaking from sample to
                # sample decays by prod(f) over a full window -> exactly 0
                # in fp32 (same argument as the time truncation).
                nc.vector.tensor_tensor_scan(
                    c_sb[:, lo: lo + HW], f_sb[:, lo: lo + HW],
                    u_sb[:, lo: lo + HW], 0.0,
                    op0=ALU.mult, op1=ALU.add,
                )

            # ---- o gate at the last timestep only ----
            opre = pss.tile([H, BL], F32, tag="sps", name="opre")
            for b in range(BL):
                # N=1 violates the fp32r ISA restrictions; run these in fp32
                nc.tensor.matmul(
                    opre[:, b:b + 1],
                    w_sb["o"][:, b * H:(b + 1) * H].bitcast(F32),
                    x_sb[:, (b + 1) * kt - 1:(b + 1) * kt].bitcast(F32),
                    start=True, stop=True,
                )
            o_sb = small.tile([H, BL], F32, tag="o", name="o_sb")
            if has_bias:
                for b in range(BL):
                    nc.scalar.activation(
                        o_sb[:, b:b + 1], opre[:, b:b + 1], AF.Sigmoid,
                        bias=bias_sb[:, 3 * BL + b: 3 * BL + b + 1],
                    )
            else:
                nc.scalar.activation(o_sb[:], opre[:], AF.Sigmoid)

            # ---- h_fin = o * tanh(c_fin); c_fin = last scan column ----
            tanh_c = small.tile([H, BL], F32, tag="tc", name="tanh_c")
            nc.scalar.activation(tanh_c[:], c_sb[:, kt - 1::kt], AF.Tanh)
            h_sb = small.tile([H, BL], F32, tag="h", name="h_sb")
            nc.vector.tensor_mul(h_sb[:], o_sb[:], tanh_c[:])

            # ---- head: z1 = tanh(fc1@h + b1); d = z[1]-z[0] via folded
            # matmul; log_softmax = ln(sigmoid(+/-d)) ----
            z1p = pss.tile([H, BL], F32, tag="sps", name="z1p")
            nc.tensor.matmul(z1p[0:FC, :], fc_sb[:, 0:FC], h_sb[:],
                             start=True, stop=True)
            z1_sb = small.tile([H, BL], F32, tag="z1", name="z1_sb")
            nc.scalar.activation(z1_sb[0:FC, :], z1p[0:FC, :], AF.Tanh,
                                 bias=fc_sb[0:FC, 32:33])
            vp = pss.tile([H, BL], F32, tag="sps", name="vp")
            nc.tensor.matmul(vp[0:O, :], fc_sb[0:FC, 33:35],
                             z1_sb[0:FC, :], start=True, stop=True)
            s_sb = small.tile([H, BL], F32, tag="s", name="s_sb")
            nc.scalar.activation(s_sb[0:O, :], vp[0:O, :], AF.Sigmoid,
                                 bias=fc_sb[0:O, 35:36])
            res = small.tile([H, BL], F32, tag="res", name="res")
            nc.scalar.activation(res[0:O, :], s_sb[0:O, :], AF.Ln)
            nc.sync.dma_start(out[:], res[0:O, :])

    nc.compile()
    return nc


def _pick_kt(inputs):
    """Smallest safe truncation window, validated on the actual inputs:
    the dropped contribution to c_fin is bounded by prod(f over window)
    * |c_before|, with |c_before| <= T (since |u_t| <= 1)."""
    x = np.asarray(inputs["x"], dtype=np.float32)
    w_f = np.asarray(inputs["w_if"], dtype=np.float32)
    b_f = np.asarray(inputs["b_f"], dtype=np.float32)[:, :, 0]
    kt = KT
    while kt < T:
        pre = np.einsum("bhi,tbi->tbh", w_f, x[-kt:]) + b_f[None]
        s = np.minimum(pre, 0.0) - np.log1p(np.exp(-np.abs(pre)))  # log sigmoid
        if s.sum(axis=0).max() < -40.0:   # err < e^-40 * 2048 ~ 1e-14
            return kt
        kt *= 2
    return T


def prepare_in_maps(inputs, kt):
    """Shard + pre-transpose the full inputs into per-core DMA-friendly maps."""
    x = np.ascontiguousarray(np.asarray(inputs["x"], dtype=np.float32)[-kt:])
    ws = {k: np.asarray(inputs[k], dtype=np.float32)
          for k in ("w_ig", "w_ii", "w_if", "w_io")}
    b_g = np.asarray(inputs["b_g"], dtype=np.float32)[:, :, 0]
    b_i = np.asarray(inputs["b_i"], dtype=np.float32)[:, :, 0]
    b_f = np.asarray(inputs["b_f"], dtype=np.float32)[:, :, 0]
    b_o = np.asarray(inputs["b_o"], dtype=np.float32)[:, :, 0]
    has_bias = any(np.any(v) for v in (b_g, b_i, b_f, b_o))

    fc2_w = np.asarray(inputs["fc2_w"], np.float32)
    fc2_b = np.asarray(inputs["fc2_b"], np.float32)
    dw = fc2_w[1] - fc2_w[0]          # [FC]
    db = np.float32(fc2_b[1] - fc2_b[0])

    fc_pack = np.zeros((H, 40), np.float32)
    fc_pack[:, 0:FC] = np.asarray(inputs["fc1_w"], np.float32).T
    fc_pack[0:FC, 32] = np.asarray(inputs["fc1_b"], np.float32)
    fc_pack[0:FC, 33] = -dw           # lhsT col 0: row 0 of V = -d
    fc_pack[0:FC, 34] = dw            # lhsT col 1: row 1 of V = +d
    fc_pack[0, 35] = -db              # sigmoid bias row 0
    fc_pack[1, 35] = db               # sigmoid bias row 1

    in_maps = []
    for c in range(NCORES):
        bs = slice(c * BL, (c + 1) * BL)
        m = {
            "xt": np.ascontiguousarray(x[:, bs, :].transpose(2, 1, 0)),
            "fc": fc_pack,
        }
        for key, name in (("w_ig", "wg"), ("w_ii", "wi"),
                          ("w_if", "wf"), ("w_io", "wo")):
            m[name] = np.ascontiguousarray(ws[key][bs].transpose(2, 0, 1))
        if has_bias:
            bp = np.zeros((H, 4 * BL), np.float32)
            for gi, bb in enumerate((b_g, b_i, b_f, b_o)):
                bp[:, gi * BL:(gi + 1) * BL] = bb[bs].T
            m["bias"] = bp
        in_maps.append(m)
    return in_maps, has_bias


_NC_CACHE = {}


def get_nc(has_bias: bool, kt: int):
    key = (has_bias, kt)
    if key not in _NC_CACHE:
        _NC_CACHE[key] = build_nc(has_bias, kt)
    return _NC_CACHE[key]


def _install_ntff_hook_shim():
    """The agent image's ``antenv`` lacks ``axon_hooks``; provide it so
    ``run_bass_kernel_spmd(trace=True)`` can reach the axon NTFF profiler."""
    import sys as _sys
    import types

    if "antenv.axon_hooks" in _sys.modules:
        return
    mod = types.ModuleType("antenv.axon_hooks")
    _state = {"hook": None}
    mod.set_axon_ntff_profile_hook = lambda h: _state.__setitem__("hook", h)
    mod.get_axon_ntff_profile_hook = lambda: _state["hook"]
    _sys.modules["antenv.axon_hooks"] = mod
    try:
        from trn_agent_boot.trn_boot import _ntff_profile_via_ctypes
        _state["hook"] = _ntff_profile_via_ctypes("/opt/axon/libaxon_pjrt.so")
    except Exception:
        pass


def kernel(**inputs) -> np.ndarray:
    from concourse.bass_utils import run_bass_kernel_spmd

    trace = os.environ.get("KERNEL_TRACE", "0") == "1"
    if trace:
        _install_ntff_hook_shim()
    kt = _pick_kt(inputs)
    in_maps, has_bias = prepare_in_maps(inputs, kt)
    nc = get_nc(has_bias, kt)
    res = run_bass_kernel_spmd(nc, in_maps, core_ids=list(range(NCORES)),
                               trace=trace)
    if res.exec_time_ns is not None:
        print(f"HW exec time: {res.exec_time_ns} ns")
    out = np.concatenate([r["out"].T for r in res.results], axis=0)
    return np.ascontiguousarray(out.astype(np.float32))


# revision 18
# speedup vs baseline: 1.2299x; 1.2299x over previous
"""
Trainium2 Bass kernel for nn_ALSTM_SIN (broken-recurrence LSTM + FC head).

Model (from the reference):
  - gate pre-activations depend ONLY on x (the hidden-state recurrence
    multiplies a zero vector, so w_h* / recurrent terms are exactly 0):
        g = tanh(w_ig[b] @ x_t + b_g),  i/f/o = sigmoid(w_i{i,f,o}[b] @ x_t + b_*)
  - cell scan: c_t = f_t * c_{t-1} + i_t * g_t  (elementwise over (B,H))
  - h_fin = o_{T-1} * tanh(c_{T-1});  out = log_softmax(tanh(h@fc1.T+b1)@fc2.T+b2)

Key numerical fact (verified against the reference, and guarded at run time
on the actual inputs): the suffix product of the forget gates over the last
KT=256 timesteps underflows fp32 for every (b,h) lane, so (a) the scan
truncated to the last KT steps is exact in fp32, and (b) chaining the 4
per-sample scans of a half-batch into one fused scan along the free dim is
also exact (the cross-sample carry decays by the same product).  Only o at
t=T-1 matters for the output.

log_softmax over 2 classes is computed as ln(sigmoid(+/-d)), d = z1 - z0,
which needs no exp / max-subtraction and keeps the ACT table-set switches
to a single Ln load at the end.

Sharding: data-parallel over the per-sample weight/batch dim: 8 samples per
core x 8 cores.  No collectives; host concatenates per-core outputs.

Gate matmuls run in float32r (TF32-like, ~13-bit mantissa, full PE rate at
moving dim >= 256); the tiny o-gate / FC-head matmuls stay fp32.

Per-core device layout (host pre-transposes so all DMAs are contiguous):
  xt [I=128, BL=8, KT]  x slice, partition = input feature i
  wg/wi/wf/wo [I=128, BL=8, H=128]   w[b].T blocks (lhsT for matmul)
  fc [128, 40]: cols 0:32 fc1_w.T | col 32 fc1_b | cols 33:35 +/-dw (lhsT
     for the +/-d matmul) | col 35 +/-db bias | col 36 scratch
  out [O=2, BL=8] = log_softmax.T (host transposes back)
"""

import os
import sys

import numpy as np

sys.path.insert(0, "/opt/trn_rl_repo")

import concourse.bacc as bacc
import concourse.bass as bass
import concourse.mybir as mybir
import concourse.tile as tile

T, B, I, H, FC, O = 2048, 64, 128, 128, 32, 2
NCORES = 8
BL = B // NCORES            # samples per core
KT = 256                    # truncated timesteps (see header note)
AF = mybir.ActivationFunctionType
ALU = mybir.AluOpType
F32 = mybir.dt.float32
F32R = mybir.dt.float32r    # fp32 bits; PE rounds to ~13-bit mantissa
BF16 = mybir.dt.bfloat16
N_WARM = 10                 # PE warm-up matmuls (HAM un-throttle)


def build_nc(has_bias: bool, kt: int = KT):
    nc = bacc.Bacc(None, target_bir_lowering=False)

    SPH = BL // 2          # samples per half
    HW = SPH * kt          # free width of one half
    WH = SPH * H           # weight cols per half

    # Host packs per-half blobs so the whole input streams as 6 ordered
    # DMAs on one queue (half-0 data lands first; issue overhead is ~0.6us
    # per dma_start on the issuing sequencer):
    #   ba{h} = [wi_h | x_h]  (first-needed), bb{h} = [wg_h | wf_h]
    ba0 = nc.dram_tensor("ba0", [I, WH + HW], F32R, kind="ExternalInput")
    bb0 = nc.dram_tensor("bb0", [I, 2 * WH], F32R, kind="ExternalInput")
    ba1 = nc.dram_tensor("ba1", [I, WH + HW], F32R, kind="ExternalInput")
    bb1 = nc.dram_tensor("bb1", [I, 2 * WH], F32R, kind="ExternalInput")
    wo = nc.dram_tensor("wo", [I, BL * H], F32R, kind="ExternalInput")
    fcp = nc.dram_tensor("fc", [H, 40], F32, kind="ExternalInput")
    bias = None
    if has_bias:
        bias = nc.dram_tensor("bias", [H, 4 * BL], F32, kind="ExternalInput")
    out = nc.dram_tensor("out", [O, BL], F32, kind="ExternalOutput")

    with tile.TileContext(nc) as tc:
        with (
            tc.tile_pool(name="inp", bufs=1) as inp,
            tc.tile_pool(name="gates", bufs=1) as gates,
            tc.tile_pool(name="small", bufs=1) as small,
            tc.tile_pool(name="psum_big", bufs=3, space="PSUM") as psb,
            tc.tile_pool(name="psum_small", bufs=2, space="PSUM") as pss,
        ):
            # ---- PE warm-up: dummy bf16 matmuls on a zero tile so the HAM
            # clock gate opens while the input DMAs stream ----
            wz = small.tile([H, 512], BF16, tag="wz", name="wz")
            nc.gpsimd.memset(wz[:], 0.0)
            for wi_ in range(N_WARM):
                wps = pss.tile([H, 512], F32, tag="sps", name="warm_ps")
                nc.tensor.matmul(wps[:], wz[:, 0:H], wz[:], start=True,
                                 stop=True)

            ba_sb = [inp.tile([I, WH + HW], F32R, tag=f"ba{h}",
                              name=f"ba{h}_sb") for h in range(2)]
            bb_sb = [inp.tile([I, 2 * WH], F32R, tag=f"bb{h}",
                              name=f"bb{h}_sb") for h in range(2)]
            wo_sb = inp.tile([I, BL * H], F32R, tag="wo", name="wo_sb")
            fc_sb = small.tile([H, 40], F32, tag="fc", name="fc_sb")
            bias_sb = None

            # All bulk DMAs on ONE queue in priority order (parallel queues
            # steal SDMA bandwidth from the critical half-0 stream).
            nc.sync.dma_start(ba_sb[0][:], ba0[:])
            nc.sync.dma_start(bb_sb[0][:], bb0[:])
            nc.sync.dma_start(ba_sb[1][:], ba1[:])
            nc.sync.dma_start(bb_sb[1][:], bb1[:])
            nc.sync.dma_start(wo_sb[:], wo[:])
            nc.sync.dma_start(fc_sb[:], fcp[:])
            if has_bias:
                bias_sb = small.tile([H, 4 * BL], F32, tag="bias",
                                     name="bias_sb")
                nc.sync.dma_start(bias_sb[:], bias[:])

            def w_slice(name, b):
                h, s = divmod(b, SPH)
                if name == "i":
                    return ba_sb[h][:, s * H:(s + 1) * H]
                if name == "g":
                    return bb_sb[h][:, s * H:(s + 1) * H]
                if name == "f":
                    return bb_sb[h][:, WH + s * H:WH + (s + 1) * H]
                return wo_sb[:, b * H:(b + 1) * H]

            def x_slice(b, c0, c1):
                h, s = divmod(b, SPH)
                return ba_sb[h][:, WH + s * kt + c0:WH + s * kt + c1]

            g_sb = gates.tile([H, BL * kt], F32, tag="g", name="g_sb")
            i_sb = gates.tile([H, BL * kt], F32, tag="i", name="i_sb")
            f_sb = gates.tile([H, BL * kt], F32, tag="f", name="f_sb")
            u_sb = gates.tile([H, BL * kt], F32, tag="u", name="u_sb")
            c_sb = gates.tile([H, BL * kt], F32, tag="c", name="c_sb")

            # Gate order (i, g, f): the first ACT op is a Sigmoid, so walrus
            # loads sigmoid_and_others (which also has Tanh) once; u = i*g
            # runs on DVE in parallel with f's activation on ACT.
            gate_cfg = [("i", i_sb, AF.Sigmoid, 1), ("g", g_sb, AF.Tanh, 0),
                        ("f", f_sb, AF.Sigmoid, 2)]
            for half in range(2):
                lo = half * HW
                for name, dst, func, gidx in gate_cfg:
                    ps = psb.tile([H, HW], F32, tag="gate_ps", name="gate_ps")
                    for s in range(SPH):
                        b = half * SPH + s
                        nc.tensor.matmul(
                            ps[:, s * kt:(s + 1) * kt],
                            w_slice(name, b),
                            x_slice(b, 0, kt),
                            start=True, stop=True,
                        )
                    if has_bias:
                        for s in range(SPH):
                            b = half * SPH + s
                            nc.scalar.activation(
                                dst[:, lo + s * kt: lo + (s + 1) * kt],
                                ps[:, s * kt:(s + 1) * kt],
                                func,
                                bias=bias_sb[:, gidx * BL + b: gidx * BL + b + 1],
                            )
                    else:
                        nc.scalar.activation(dst[:, lo: lo + HW], ps[:], func)
                    if name == "g":
                        nc.vector.tensor_mul(
                            u_sb[:, lo: lo + HW],
                            i_sb[:, lo: lo + HW],
                            g_sb[:, lo: lo + HW],
                        )
                # One fused scan per half: the carry leaking from sample to
                # sample decays by prod(f) over a full window -> exactly 0
                # in fp32 (same argument as the time truncation).
                nc.vector.tensor_tensor_scan(
                    c_sb[:, lo: lo + HW], f_sb[:, lo: lo + HW],
                    u_sb[:, lo: lo + HW], 0.0,
                    op0=ALU.mult, op1=ALU.add,
                )

            # ---- o gate at the last timestep only ----
            opre = pss.tile([H, BL], F32, tag="sps", name="opre")
            for b in range(BL):
                # N=1 violates the fp32r ISA restrictions; run these in fp32
                nc.tensor.matmul(
                    opre[:, b:b + 1],
                    w_slice("o", b).bitcast(F32),
                    x_slice(b, kt - 1, kt).bitcast(F32),
                    start=True, stop=True,
                )
            o_sb = small.tile([H, BL], F32, tag="o", name="o_sb")
            if has_bias:
                for b in range(BL):
                    nc.scalar.activation(
                        o_sb[:, b:b + 1], opre[:, b:b + 1], AF.Sigmoid,
                        bias=bias_sb[:, 3 * BL + b: 3 * BL + b + 1],
                    )
            else:
                nc.scalar.activation(o_sb[:], opre[:], AF.Sigmoid)

            # ---- h_fin = o * tanh(c_fin); c_fin = last scan column ----
            tanh_c = small.tile([H, BL], F32, tag="tc", name="tanh_c")
            nc.scalar.activation(tanh_c[:], c_sb[:, kt - 1::kt], AF.Tanh)
            h_sb = small.tile([H, BL], F32, tag="h", name="h_sb")
            nc.vector.tensor_mul(h_sb[:], o_sb[:], tanh_c[:])

            # ---- head: z1 = tanh(fc1@h + b1); d = z[1]-z[0] via folded
            # matmul; log_softmax = ln(sigmoid(+/-d)) ----
            z1p = pss.tile([H, BL], F32, tag="sps", name="z1p")
            nc.tensor.matmul(z1p[0:FC, :], fc_sb[:, 0:FC], h_sb[:],
                             start=True, stop=True)
            z1_sb = small.tile([H, BL], F32, tag="z1", name="z1_sb")
            nc.scalar.activation(z1_sb[0:FC, :], z1p[0:FC, :], AF.Tanh,
                                 bias=fc_sb[0:FC, 32:33])
            vp = pss.tile([H, BL], F32, tag="sps", name="vp")
            nc.tensor.matmul(vp[0:O, :], fc_sb[0:FC, 33:35],
                             z1_sb[0:FC, :], start=True, stop=True)
            s_sb = small.tile([H, BL], F32, tag="s", name="s_sb")
            nc.scalar.activation(s_sb[0:O, :], vp[0:O, :], AF.Sigmoid,
                                 bias=fc_sb[0:O, 35:36])
            res = small.tile([H, BL], F32, tag="res", name="res")
            nc.scalar.activation(res[0:O, :], s_sb[0:O, :], AF.Ln)
            nc.sync.dma_start(out[:], res[0:O, :])

    nc.compile()
    return nc


def _pick_kt(inputs):
    """Smallest safe truncation window, validated on the actual inputs:
    the dropped contribution to c_fin is bounded by prod(f over window)
    * |c_before|, with |c_before| <= T (since |u_t| <= 1)."""
    x = np.asarray(inputs["x"], dtype=np.float32)
    w_f = np.asarray(inputs["w_if"], dtype=np.float32)
    b_f = np.asarray(inputs["b_f"], dtype=np.float32)[:, :, 0]
    kt = KT
    while kt < T:
        pre = np.einsum("bhi,tbi->tbh", w_f, x[-kt:]) + b_f[None]
        s = np.minimum(pre, 0.0) - np.log1p(np.exp(-np.abs(pre)))  # log sigmoid
        if s.sum(axis=0).max() < -40.0:   # err < e^-40 * 2048 ~ 1e-14
            return kt
        kt *= 2
    return T


def prepare_in_maps(inputs, kt):
    """Shard + pre-transpose the full inputs into per-core DMA-friendly maps."""
    x = np.ascontiguousarray(np.asarray(inputs["x"], dtype=np.float32)[-kt:])
    ws = {k: np.asarray(inputs[k], dtype=np.float32)
          for k in ("w_ig", "w_ii", "w_if", "w_io")}
    b_g = np.asarray(inputs["b_g"], dtype=np.float32)[:, :, 0]
    b_i = np.asarray(inputs["b_i"], dtype=np.float32)[:, :, 0]
    b_f = np.asarray(inputs["b_f"], dtype=np.float32)[:, :, 0]
    b_o = np.asarray(inputs["b_o"], dtype=np.float32)[:, :, 0]
    has_bias = any(np.any(v) for v in (b_g, b_i, b_f, b_o))

    fc2_w = np.asarray(inputs["fc2_w"], np.float32)
    fc2_b = np.asarray(inputs["fc2_b"], np.float32)
    dw = fc2_w[1] - fc2_w[0]          # [FC]
    db = np.float32(fc2_b[1] - fc2_b[0])

    fc_pack = np.zeros((H, 40), np.float32)
    fc_pack[:, 0:FC] = np.asarray(inputs["fc1_w"], np.float32).T
    fc_pack[0:FC, 32] = np.asarray(inputs["fc1_b"], np.float32)
    fc_pack[0:FC, 33] = -dw           # lhsT col 0: row 0 of V = -d
    fc_pack[0:FC, 34] = dw            # lhsT col 1: row 1 of V = +d
    fc_pack[0, 35] = -db              # sigmoid bias row 0
    fc_pack[1, 35] = db               # sigmoid bias row 1

    SPH = BL // 2
    in_maps = []
    for c in range(NCORES):
        bs = slice(c * BL, (c + 1) * BL)
        # per-core [I, b, ...] views
        xc = x[:, bs, :].transpose(2, 1, 0)              # [I, BL, kt]
        wc = {k: ws[k][bs].transpose(2, 0, 1)            # [I, BL, H]
              for k in ("w_ig", "w_ii", "w_if", "w_io")}
        m = {"fc": fc_pack,
             "wo": np.ascontiguousarray(
                 wc["w_io"].reshape(H, BL * H))}
        for h in range(2):
            sb = slice(h * SPH, (h + 1) * SPH)
            ba = np.concatenate(
                [wc["w_ii"][:, sb].reshape(H, SPH * H),
                 xc[:, sb].reshape(H, SPH * kt)], axis=1)
            bb = np.concatenate(
                [wc["w_ig"][:, sb].reshape(H, SPH * H),
                 wc["w_if"][:, sb].reshape(H, SPH * H)], axis=1)
            m[f"ba{h}"] = np.ascontiguousarray(ba)
            m[f"bb{h}"] = np.ascontiguousarray(bb)
        if has_bias:
            bp = np.zeros((H, 4 * BL), np.float32)
            for gi, bb_ in enumerate((b_g, b_i, b_f, b_o)):
                bp[:, gi * BL:(gi + 1) * BL] = bb_[bs].T
            m["bias"] = bp
        in_maps.append(m)
    return in_maps, has_bias


_NC_CACHE = {}


def get_nc(has_bias: bool, kt: int):
    key = (has_bias, kt)
    if key not in _NC_CACHE:
        _NC_CACHE[key] = build_nc(has_bias, kt)
    return _NC_CACHE[key]


def _install_ntff_hook_shim():
    """The agent image's ``antenv`` lacks ``axon_hooks``; provide it so
    ``run_bass_kernel_spmd(trace=True)`` can reach the axon NTFF profiler."""
    import sys as _sys
    import types

    if "antenv.axon_hooks" in _sys.modules:
        return
    mod = types.ModuleType("antenv.axon_hooks")
    _state = {"hook": None}
    mod.set_axon_ntff_profile_hook = lambda h: _state.__setitem__("hook", h)
    mod.get_axon_ntff_profile_hook = lambda: _state["hook"]
    _sys.modules["antenv.axon_hooks"] = mod
    try:
        from trn_agent_boot.trn_boot import _ntff_profile_via_ctypes
        _state["hook"] = _ntff_profile_via_ctypes("/opt/axon/libaxon_pjrt.so")
    except Exception:
        pass


def kernel(**inputs) -> np.ndarray:
    from concourse.bass_utils import run_bass_kernel_spmd

    trace = os.environ.get("KERNEL_TRACE", "0") == "1"
    if trace:
        _install_ntff_hook_shim()
    kt = _pick_kt(inputs)
    in_maps, has_bias = prepare_in_maps(inputs, kt)
    nc = get_nc(has_bias, kt)
    res = run_bass_kernel_spmd(nc, in_maps, core_ids=list(range(NCORES)),
                               trace=trace)
    if res.exec_time_ns is not None:
        print(f"HW exec time: {res.exec_time_ns} ns")
    out = np.concatenate([r["out"].T for r in res.results], axis=0)
    return np.ascontiguousarray(out.astype(np.float32))


# revision 27
# speedup vs baseline: 1.2658x; 1.0292x over previous
"""
Trainium2 Bass kernel for nn_ALSTM_SIN (broken-recurrence LSTM + FC head).

Model (from the reference):
  - gate pre-activations depend ONLY on x (the hidden-state recurrence
    multiplies a zero vector, so w_h* / recurrent terms are exactly 0):
        g = tanh(w_ig[b] @ x_t + b_g),  i/f/o = sigmoid(w_i{i,f,o}[b] @ x_t + b_*)
  - cell scan: c_t = f_t * c_{t-1} + i_t * g_t  (elementwise over (B,H))
  - h_fin = o_{T-1} * tanh(c_{T-1});  out = log_softmax(tanh(h@fc1.T+b1)@fc2.T+b2)

Key numerical fact (verified against the reference, and guarded at run time
on the actual inputs): the suffix product of the forget gates over the last
KT=256 timesteps underflows fp32 for every (b,h) lane, so (a) the scan
truncated to the last KT steps is exact in fp32, and (b) chaining the 4
per-sample scans of a half-batch into one fused scan along the free dim is
also exact (the cross-sample carry decays by the same product).  Only o at
t=T-1 matters for the output.

log_softmax over 2 classes is computed as ln(sigmoid(+/-d)), d = z1 - z0,
which needs no exp / max-subtraction and keeps the ACT table-set switches
to a single Ln load at the end.

Sharding: data-parallel over the per-sample weight/batch dim: 8 samples per
core x 8 cores.  No collectives; host concatenates per-core outputs.

Gate inputs ship as fp16 (the PE's fp32r path keeps only ~10 mantissa
bits anyway, so this costs no accuracy but halves the DMA stream and runs
matmuls at full 1 cyc/row); the tiny FC-head matmuls stay fp32.

Per-core device layout (host pre-packs so all DMAs are contiguous and
stream in dependency order on one queue):
  ba{h} [I=128, 4*H + 4*KT] = [w_ii[b].T blocks | x blocks] for half h
  wg{h}/wf{h} [I, 4*H] = w_ig/w_if[b].T blocks;  wo [I, 8*H]
  fc [128, 40]: cols 0:32 fc1_w.T | col 32 fc1_b | cols 33:35 +/-dw (lhsT
     for the +/-d matmul) | col 35 +/-db bias
  out [O=2, BL=8] = log_softmax.T (host transposes back)
"""

import os
import sys

import numpy as np

sys.path.insert(0, "/opt/trn_rl_repo")

import concourse.bacc as bacc
import concourse.mybir as mybir
import concourse.tile as tile

T, B, I, H, FC, O = 2048, 64, 128, 128, 32, 2
NCORES = 8
BL = B // NCORES            # samples per core
KT = 256                    # truncated timesteps (see header note)
AF = mybir.ActivationFunctionType
ALU = mybir.AluOpType
F32 = mybir.dt.float32
F16 = mybir.dt.float16      # gate inputs: PE fp32r keeps only ~10 mantissa
                            # bits anyway, so fp16 inputs cost no accuracy
                            # but halve the DMA stream and run 1 cyc/row
BF16 = mybir.dt.bfloat16
N_WARM = 10                 # PE warm-up matmuls (HAM un-throttle)


def build_nc(has_bias: bool, kt: int = KT):
    nc = bacc.Bacc(None, target_bir_lowering=False)

    SPH = BL // 2          # samples per half
    HW = SPH * kt          # free width of one half
    WH = SPH * H           # weight cols per half

    # Host packs per-half blobs so the whole input streams as 8 ordered
    # DMAs on one queue (half-0 data lands first; issue overhead is ~0.6us
    # per dma_start on the issuing sequencer): ba{h} = [wi_h | x_h]
    ba0 = nc.dram_tensor("ba0", [I, WH + HW], F16, kind="ExternalInput")
    wg0d = nc.dram_tensor("wg0", [I, WH], F16, kind="ExternalInput")
    wf0d = nc.dram_tensor("wf0", [I, WH], F16, kind="ExternalInput")
    ba1 = nc.dram_tensor("ba1", [I, WH + HW], F16, kind="ExternalInput")
    wg1d = nc.dram_tensor("wg1", [I, WH], F16, kind="ExternalInput")
    wf1d = nc.dram_tensor("wf1", [I, WH], F16, kind="ExternalInput")
    wo = nc.dram_tensor("wo", [I, BL * H], F16, kind="ExternalInput")
    fcp = nc.dram_tensor("fc", [H, 40], F32, kind="ExternalInput")
    bias = None
    if has_bias:
        bias = nc.dram_tensor("bias", [H, 4 * BL], F32, kind="ExternalInput")
    out = nc.dram_tensor("out", [O, BL], F32, kind="ExternalOutput")

    with tile.TileContext(nc) as tc:
        with (
            tc.tile_pool(name="inp", bufs=1) as inp,
            tc.tile_pool(name="gates", bufs=1) as gates,
            tc.tile_pool(name="small", bufs=1) as small,
            tc.tile_pool(name="psum_big", bufs=3, space="PSUM") as psb,
            tc.tile_pool(name="psum_small", bufs=2, space="PSUM") as pss,
        ):
            # ---- PE warm-up: dummy bf16 matmuls on a zero tile so the HAM
            # clock gate opens while the input DMAs stream ----
            wz = small.tile([H, 512], BF16, tag="wz", name="wz")
            nc.gpsimd.memset(wz[:], 0.0)
            for wi_ in range(N_WARM):
                wps = pss.tile([H, 512], F32, tag="sps", name="warm_ps")
                nc.tensor.matmul(wps[:], wz[:, 0:H], wz[:], start=True,
                                 stop=True)

            ba_sb = [inp.tile([I, WH + HW], F16, tag=f"ba{h}",
                              name=f"ba{h}_sb") for h in range(2)]
            wg_sb = [inp.tile([I, WH], F16, tag=f"wg{h}",
                              name=f"wg{h}_sb") for h in range(2)]
            wf_sb = [inp.tile([I, WH], F16, tag=f"wf{h}",
                              name=f"wf{h}_sb") for h in range(2)]
            wo_sb = inp.tile([I, BL * H], F16, tag="wo", name="wo_sb")
            fc_sb = small.tile([H, 40], F32, tag="fc", name="fc_sb")
            bias_sb = None

            # All bulk DMAs on ONE queue in priority order (parallel queues
            # steal SDMA bandwidth from the critical half-0 stream).
            nc.sync.dma_start(ba_sb[0][:], ba0[:])
            nc.sync.dma_start(wg_sb[0][:], wg0d[:])
            nc.sync.dma_start(wf_sb[0][:], wf0d[:])
            nc.sync.dma_start(ba_sb[1][:], ba1[:])
            nc.sync.dma_start(wg_sb[1][:], wg1d[:])
            nc.sync.dma_start(wf_sb[1][:], wf1d[:])
            nc.sync.dma_start(wo_sb[:], wo[:])
            nc.sync.dma_start(fc_sb[:], fcp[:])
            if has_bias:
                bias_sb = small.tile([H, 4 * BL], F32, tag="bias",
                                     name="bias_sb")
                nc.sync.dma_start(bias_sb[:], bias[:])

            def w_slice(name, b):
                h, s = divmod(b, SPH)
                if name == "i":
                    return ba_sb[h][:, s * H:(s + 1) * H]
                if name == "g":
                    return wg_sb[h][:, s * H:(s + 1) * H]
                if name == "f":
                    return wf_sb[h][:, s * H:(s + 1) * H]
                return wo_sb[:, b * H:(b + 1) * H]

            def x_slice(b, c0, c1):
                h, s = divmod(b, SPH)
                return ba_sb[h][:, WH + s * kt + c0:WH + s * kt + c1]

            g_sb = gates.tile([H, BL * kt], F32, tag="g", name="g_sb")
            i_sb = gates.tile([H, BL * kt], F32, tag="i", name="i_sb")
            f_sb = gates.tile([H, BL * kt], F32, tag="f", name="f_sb")
            u_sb = gates.tile([H, BL * kt], F32, tag="u", name="u_sb")
            c_sb = gates.tile([H, BL * kt], F32, tag="c", name="c_sb")

            # Gate order (i, g, f): the first ACT op is a Sigmoid, so walrus
            # loads sigmoid_and_others (which also has Tanh) once; u = i*g
            # runs on DVE in parallel with f's activation on ACT.
            gate_cfg = [("i", i_sb, AF.Sigmoid, 1), ("g", g_sb, AF.Tanh, 0),
                        ("f", f_sb, AF.Sigmoid, 2)]
            for half in range(2):
                lo = half * HW
                for name, dst, func, gidx in gate_cfg:
                    ps = psb.tile([H, HW], F32, tag="gate_ps", name="gate_ps")
                    for s in range(SPH):
                        b = half * SPH + s
                        nc.tensor.matmul(
                            ps[:, s * kt:(s + 1) * kt],
                            w_slice(name, b),
                            x_slice(b, 0, kt),
                            start=True, stop=True,
                        )
                    if has_bias:
                        for s in range(SPH):
                            b = half * SPH + s
                            nc.scalar.activation(
                                dst[:, lo + s * kt: lo + (s + 1) * kt],
                                ps[:, s * kt:(s + 1) * kt],
                                func,
                                bias=bias_sb[:, gidx * BL + b: gidx * BL + b + 1],
                            )
                    else:
                        nc.scalar.activation(dst[:, lo: lo + HW], ps[:], func)
                    if name == "g":
                        nc.vector.tensor_mul(
                            u_sb[:, lo: lo + HW],
                            i_sb[:, lo: lo + HW],
                            g_sb[:, lo: lo + HW],
                        )
                # One fused scan per half: the carry leaking from sample to
                # sample decays by prod(f) over a full window -> exactly 0
                # in fp32 (same argument as the time truncation).
                nc.vector.tensor_tensor_scan(
                    c_sb[:, lo: lo + HW], f_sb[:, lo: lo + HW],
                    u_sb[:, lo: lo + HW], 0.0,
                    op0=ALU.mult, op1=ALU.add,
                )

            # ---- o gate at the last timestep only ----
            opre = pss.tile([H, BL], F32, tag="sps", name="opre")
            for b in range(BL):
                nc.tensor.matmul(
                    opre[:, b:b + 1],
                    w_slice("o", b),
                    x_slice(b, kt - 1, kt),
                    start=True, stop=True,
                )
            o_sb = small.tile([H, BL], F32, tag="o", name="o_sb")
            if has_bias:
                for b in range(BL):
                    nc.scalar.activation(
                        o_sb[:, b:b + 1], opre[:, b:b + 1], AF.Sigmoid,
                        bias=bias_sb[:, 3 * BL + b: 3 * BL + b + 1],
                    )
            else:
                nc.scalar.activation(o_sb[:], opre[:], AF.Sigmoid)

            # ---- h_fin = o * tanh(c_fin); c_fin = last scan column ----
            tanh_c = small.tile([H, BL], F32, tag="tc", name="tanh_c")
            nc.scalar.activation(tanh_c[:], c_sb[:, kt - 1::kt], AF.Tanh)
            h_sb = small.tile([H, BL], F32, tag="h", name="h_sb")
            nc.vector.tensor_mul(h_sb[:], o_sb[:], tanh_c[:])

            # ---- head: z1 = tanh(fc1@h + b1); d = z[1]-z[0] via folded
            # matmul; log_softmax = ln(sigmoid(+/-d)) ----
            z1p = pss.tile([H, BL], F32, tag="sps", name="z1p")
            nc.tensor.matmul(z1p[0:FC, :], fc_sb[:, 0:FC], h_sb[:],
                             start=True, stop=True)
            z1_sb = small.tile([H, BL], F32, tag="z1", name="z1_sb")
            nc.scalar.activation(z1_sb[0:FC, :], z1p[0:FC, :], AF.Tanh,
                                 bias=fc_sb[0:FC, 32:33])
            vp = pss.tile([H, BL], F32, tag="sps", name="vp")
            nc.tensor.matmul(vp[0:O, :], fc_sb[0:FC, 33:35],
                             z1_sb[0:FC, :], start=True, stop=True)
            s_sb = small.tile([H, BL], F32, tag="s", name="s_sb")
            nc.scalar.activation(s_sb[0:O, :], vp[0:O, :], AF.Sigmoid,
                                 bias=fc_sb[0:O, 35:36])
            res = small.tile([H, BL], F32, tag="res", name="res")
            nc.scalar.activation(res[0:O, :], s_sb[0:O, :], AF.Ln)
            nc.sync.dma_start(out[:], res[0:O, :])

    nc.compile()
    return nc


def _pick_kt(inputs):
    """Smallest safe truncation window, validated on the actual inputs:
    the dropped contribution to c_fin is bounded by prod(f over window)
    * |c_before|, with |c_before| <= T (since |u_t| <= 1)."""
    x = np.asarray(inputs["x"], dtype=np.float32)
    w_f = np.asarray(inputs["w_if"], dtype=np.float32)
    b_f = np.asarray(inputs["b_f"], dtype=np.float32)[:, :, 0]
    kt = KT
    while kt < T:
        pre = np.einsum("bhi,tbi->tbh", w_f, x[-kt:]) + b_f[None]
        s = np.minimum(pre, 0.0) - np.log1p(np.exp(-np.abs(pre)))  # log sigmoid
        if s.sum(axis=0).max() < -40.0:   # err < e^-40 * 2048 ~ 1e-14
            return kt
        kt *= 2
    return T


def prepare_in_maps(inputs, kt):
    """Shard + pre-transpose the full inputs into per-core DMA-friendly maps."""
    x = np.ascontiguousarray(np.asarray(inputs["x"], dtype=np.float32)[-kt:])
    ws = {k: np.asarray(inputs[k], dtype=np.float32)
          for k in ("w_ig", "w_ii", "w_if", "w_io")}
    b_g = np.asarray(inputs["b_g"], dtype=np.float32)[:, :, 0]
    b_i = np.asarray(inputs["b_i"], dtype=np.float32)[:, :, 0]
    b_f = np.asarray(inputs["b_f"], dtype=np.float32)[:, :, 0]
    b_o = np.asarray(inputs["b_o"], dtype=np.float32)[:, :, 0]
    has_bias = any(np.any(v) for v in (b_g, b_i, b_f, b_o))

    fc2_w = np.asarray(inputs["fc2_w"], np.float32)
    fc2_b = np.asarray(inputs["fc2_b"], np.float32)
    dw = fc2_w[1] - fc2_w[0]          # [FC]
    db = np.float32(fc2_b[1] - fc2_b[0])

    fc_pack = np.zeros((H, 40), np.float32)
    fc_pack[:, 0:FC] = np.asarray(inputs["fc1_w"], np.float32).T
    fc_pack[0:FC, 32] = np.asarray(inputs["fc1_b"], np.float32)
    fc_pack[0:FC, 33] = -dw           # lhsT col 0: row 0 of V = -d
    fc_pack[0:FC, 34] = dw            # lhsT col 1: row 1 of V = +d
    fc_pack[0, 35] = -db              # sigmoid bias row 0
    fc_pack[1, 35] = db               # sigmoid bias row 1

    SPH = BL // 2
    in_maps = []
    for c in range(NCORES):
        bs = slice(c * BL, (c + 1) * BL)
        # per-core [I, b, ...] views
        xc = x[:, bs, :].transpose(2, 1, 0).astype(np.float16)
        wc = {k: ws[k][bs].transpose(2, 0, 1).astype(np.float16)
              for k in ("w_ig", "w_ii", "w_if", "w_io")}
        m = {"fc": fc_pack,
             "wo": np.ascontiguousarray(
                 wc["w_io"].reshape(H, BL * H))}
        for h in range(2):
            sb = slice(h * SPH, (h + 1) * SPH)
            ba = np.concatenate(
                [wc["w_ii"][:, sb].reshape(H, SPH * H),
                 xc[:, sb].reshape(H, SPH * kt)], axis=1)
            m[f"ba{h}"] = np.ascontiguousarray(ba)
            m[f"wg{h}"] = np.ascontiguousarray(
                wc["w_ig"][:, sb].reshape(H, SPH * H))
            m[f"wf{h}"] = np.ascontiguousarray(
                wc["w_if"][:, sb].reshape(H, SPH * H))
        if has_bias:
            bp = np.zeros((H, 4 * BL), np.float32)
            for gi, bb_ in enumerate((b_g, b_i, b_f, b_o)):
                bp[:, gi * BL:(gi + 1) * BL] = bb_[bs].T
            m["bias"] = bp
        in_maps.append(m)
    return in_maps, has_bias


_NC_CACHE = {}


def get_nc(has_bias: bool, kt: int):
    key = (has_bias, kt)
    if key not in _NC_CACHE:
        _NC_CACHE[key] = build_nc(has_bias, kt)
    return _NC_CACHE[key]


def _install_ntff_hook_shim():
    """The agent image's ``antenv`` lacks ``axon_hooks``; provide it so
    ``run_bass_kernel_spmd(trace=True)`` can reach the axon NTFF profiler."""
    import sys as _sys
    import types

    if "antenv.axon_hooks" in _sys.modules:
        return
    mod = types.ModuleType("antenv.axon_hooks")
    _state = {"hook": None}
    mod.set_axon_ntff_profile_hook = lambda h: _state.__setitem__("hook", h)
    mod.get_axon_ntff_profile_hook = lambda: _state["hook"]
    _sys.modules["antenv.axon_hooks"] = mod
    try:
        from trn_agent_boot.trn_boot import _ntff_profile_via_ctypes
        _state["hook"] = _ntff_profile_via_ctypes("/opt/axon/libaxon_pjrt.so")
    except Exception:
        pass


def _numpy_exact(inputs):
    """Full-length fp32 host fallback, used ONLY if the runtime truncation
    guard fails (impossible for randn-style inputs; safety net against
    pathological forget gates the device build doesn't support)."""
    x = np.asarray(inputs["x"], np.float32)
    sig = lambda z: (1.0 / (1.0 + np.exp(-z))).astype(np.float32)
    pre = lambda w: np.einsum("bhi,tbi->tbh",
                              np.asarray(inputs[w], np.float32), x)
    bias = {k: np.asarray(inputs[k], np.float32)[:, :, 0]
            for k in ("b_g", "b_i", "b_f", "b_o")}
    g = np.tanh(pre("w_ig") + bias["b_g"]).astype(np.float32)
    i_ = sig(pre("w_ii") + bias["b_i"])
    f = sig(pre("w_if") + bias["b_f"])
    u = (i_ * g).astype(np.float32)
    c = np.zeros((B, H), np.float32)
    for t in range(x.shape[0]):
        c = (f[t] * c + u[t]).astype(np.float32)
    o = sig(np.einsum("bhi,bi->bh", np.asarray(inputs["w_io"], np.float32),
                      x[-1]) + bias["b_o"])
    h = (o * np.tanh(c)).astype(np.float32)
    z1 = np.tanh(h @ np.asarray(inputs["fc1_w"], np.float32).T
                 + np.asarray(inputs["fc1_b"], np.float32)).astype(np.float32)
    z = (z1 @ np.asarray(inputs["fc2_w"], np.float32).T
         + np.asarray(inputs["fc2_b"], np.float32)).astype(np.float32)
    m = z.max(axis=1, keepdims=True)
    ls = z - (m + np.log(np.exp(z - m).sum(axis=1, keepdims=True)))
    return np.ascontiguousarray(ls.astype(np.float32))


def kernel(**inputs) -> np.ndarray:
    from concourse.bass_utils import run_bass_kernel_spmd

    trace = os.environ.get("KERNEL_TRACE", "0") == "1"
    if trace:
        _install_ntff_hook_shim()
    kt = _pick_kt(inputs)
    if kt > KT:
        print(f"WARNING: forget-gate decay guard demanded kt={kt}; "
              "falling back to exact host computation")
        return _numpy_exact(inputs)
    in_maps, has_bias = prepare_in_maps(inputs, kt)
    nc = get_nc(has_bias, kt)
    res = run_bass_kernel_spmd(nc, in_maps, core_ids=list(range(NCORES)),
                               trace=trace)
    if res.exec_time_ns is not None:
        print(f"HW exec time: {res.exec_time_ns} ns")
    out = np.concatenate([r["out"].T for r in res.results], axis=0)
    return np.ascontiguousarray(out.astype(np.float32))
